# revision 11
# baseline (speedup 1.0000x reference)
"""Trainium2 Bass kernel for nn_MultiHeadAttentionLayer (GNN message passing).

Contract: kernel(**inputs) takes the FULL unsharded inputs (as produced by
setup_inputs()) and returns the FULL output [N, H, D] float32.

Strategy (8 NeuronCores, SPMD, no collectives):
  - dst == arange(E) % N, so node n receives exactly 8 edges e = k*N + n.
    Destination nodes are sharded across cores; each core owns N/8 nodes and
    all 8 incoming edges per node, so segment_sum is a dense 8-step PSUM
    accumulation (no scatter).
  - Each core renumbers the src ids of its edges into a compacted table of
    < 32768 unique rows (int16 gather indices).  On device it projects that
    node set through [Wk|Wv] into a 768B-per-row bf16 [K|V] table in DRAM,
    laid out "wrapped" ([128, C, 384]; row r at partition r%128, chunk
    r//128) so table stores batch 12 row-tiles into a single DMA.
  - V columns are stored head-minor ((f,h) order) so the per-edge score
    multiply has a packed innermost dim and runs in the DVE 2x mode with a
    stride-0 broadcast of the per-(edge,head) weight vector; output heads are
    un-transposed on the host for free.
  - Per (node-block, 2-slice chunk): one 768B-row dma_gather, score =
    clip(exp(dot(K,Q)*scale)) (exp-then-clip == clip-then-exp by
    monotonicity, so no pre-clip pass), weight vector w = [exp-score | jac],
    scl = V * w, and identity-matmul PSUM accumulation over the 8 slices.
    Work is spread: DVE (prod/reduce/scale), ACT (exp, broadcast fills),
    Pool (gathers, clip, z-accum, final divide), PE (accumulate).

Host-side work is limited to layout (transpose/pad/unique/renumber tables)
and the final concatenation; all FLOPs happen on device.
"""

import sys

import numpy as np

for _p in ("/opt/trn_rl_repo",):
    if _p not in sys.path:
        sys.path.insert(0, _p)

# --- problem constants (hardcoded per spec; kernel.py must be self-contained)
N_NODES = 50000
N_EDGES = 400000
IN_DIM = 256
OUT_DIM = 32
NUM_HEADS = 8
H2 = NUM_HEADS // 2
N_CORES = 8
P = 128

SCALE = float(1.0 / np.sqrt(np.float32(32.0)))
CLIP_LO = float(np.exp(np.float32(-5.0)))
CLIP_HI = float(np.exp(np.float32(5.0)))
MAX_TBLC = 32256  # compacted-table cap (int16 indices; 252 * 128)


class Cfg:
    def __init__(self, n_nodes=N_NODES, n_edges=N_EDGES, n_cores=N_CORES):
        assert n_edges == 8 * n_nodes
        self.N = n_nodes
        self.E = n_edges
        self.NC = n_cores
        assert n_nodes % n_cores == 0
        self.NPC = n_nodes // n_cores          # nodes per core
        self.M = -(-self.NPC // P)             # 128-node tiles per core
        self.NPAD = self.M * P                 # padded nodes per core
        assert self.NPAD % 16 == 0
        self.W = self.NPAD // 16               # idx16 words per slice
        self.BM = 7 if self.M % 7 == 0 else 1  # m-tiles per node block
        self.NB = self.M // self.BM            # node blocks
        # compacted table rows (upper bound on per-core unique src count)
        self.TBLC = min(-(-n_nodes // 512) * 512, MAX_TBLC)
        assert self.TBLC % P == 0
        self.C = self.TBLC // P                # wrapped-table chunks
        self.TGRP = 12 if self.C % 12 == 0 else (4 if self.C % 4 == 0 else 1)
        self.NTG = self.C // self.TGRP         # table store groups


FULL_CFG = Cfg()


# --------------------------------------------------------------------------
# device program
# --------------------------------------------------------------------------

def build_program(cfg: Cfg, repeat: int = 1, ablate: str = ""):
    import concourse.bacc as bacc
    import concourse.mybir as mybir
    import concourse.tile as tile
    from concourse.masks import make_identity

    f32 = mybir.dt.float32
    bf16 = mybir.dt.bfloat16
    i16 = mybir.dt.int16
    M, BM, NB = cfg.M, cfg.BM, cfg.NB
    TGRP, NTG = cfg.TGRP, cfg.NTG

    nc = bacc.Bacc(
        "TRN2",
        target_bir_lowering=False,
        debug=False,
        enable_asserts=False,
        num_devices=cfg.NC,
    )

    # hT_tbl wrapped: [p, chunk, half, node] (see host_prepare)
    hT_tbl = nc.dram_tensor("hT_tbl", [P, cfg.C, 2, P], bf16, kind="ExternalInput")
    hT_own = nc.dram_tensor("hT_own", [IN_DIM, cfg.NPAD], bf16, kind="ExternalInput")
    w_kv = nc.dram_tensor("w_kv", [P, 2, 384], bf16, kind="ExternalInput")
    w_q = nc.dram_tensor("w_q", [P, 2, P], bf16, kind="ExternalInput")
    idx_t = nc.dram_tensor("idx", [P, 8 * cfg.W], i16, kind="ExternalInput")
    jac_t = nc.dram_tensor("jac", [P, 8 * M], bf16, kind="ExternalInput")
    out_t = nc.dram_tensor("out", [P, M * 256], f32, kind="ExternalOutput")

    mult = mybir.AluOpType.mult
    add = mybir.AluOpType.add


    with tile.TileContext(nc) as tc:
        with (
            tc.tile_pool(name="dram", bufs=2, space="DRAM") as dram,
            tc.tile_pool(name="const", bufs=1) as const,
        ):
          for _rep in range(repeat):
            kv_table = dram.tile([cfg.TBLC, 384], bf16)
            kv_w = kv_table[:].rearrange("(p c) f -> p c f", p=P)  # wrapped view

            wkv_sb = const.tile([P, 2, 384], bf16)
            wq_sb = const.tile([P, 2, P], bf16)
            idx_sb = const.tile([P, 8 * cfg.W], i16)
            jac_sb = const.tile([P, 8 * M], bf16)
            qloc = const.tile([P, M, P], bf16)
            z47r = const.tile([P, M], f32)
            ident = const.tile([P, P], bf16)

            nc.sync.dma_start(out=wkv_sb[:], in_=w_kv[:])
            nc.sync.dma_start(out=wq_sb[:], in_=w_q[:])
            nc.sync.dma_start(out=idx_sb[:], in_=idx_t[:])
            nc.sync.dma_start(out=jac_sb[:], in_=jac_t[:])
            make_identity(nc, ident[:])

            # z for jaccard heads: one strided reduce over the 8 slices
            # jac layout is block-major: [p, (b, k, m)]
            nc.vector.tensor_reduce(
                out=z47r[:].rearrange("p (b m) -> p b m", m=BM),
                in_=jac_sb[:].rearrange("p (b k m) -> p b m k", k=8, m=BM),
                axis=mybir.AxisListType.X, op=add)
            nc.vector.reciprocal(z47r[:], z47r[:])

            def _copy(i, out, in_):
                # PSUM -> SBUF eviction; gpsimd cannot read PSUM on HW
                if i % 2 == 0:
                    nc.scalar.copy(out=out, in_=in_)
                else:
                    nc.vector.tensor_copy(out=out, in_=in_)

            # ---- phase A: Q projection of own shard -----------------------
            with (
                tc.tile_pool(name="pa", bufs=1) as pa,
                tc.tile_pool(name="pa_ps", bufs=2, space="PSUM") as pap,
            ):
                hto = pa.tile([P, 2, cfg.NPAD], bf16)
                nc.sync.dma_start(out=hto[:, 0, :], in_=hT_own[0:P, :])
                nc.sync.dma_start(out=hto[:, 1, :], in_=hT_own[P:2 * P, :])
                for t in range(M):
                    ps_q = pap.tile([P, P], f32)
                    nc.tensor.matmul(
                        out=ps_q[:], lhsT=hto[:, 0, t * P:(t + 1) * P],
                        rhs=wq_sb[:, 0, :], start=True, stop=False,
                    )
                    nc.tensor.matmul(
                        out=ps_q[:], lhsT=hto[:, 1, t * P:(t + 1) * P],
                        rhs=wq_sb[:, 1, :], start=False, stop=True,
                    )
                    _copy(t, out=qloc[:, t, :], in_=ps_q[:])

            # ---- phase T: compacted K|V bf16 table projection -------------
            # 12 row-tiles per load/store; stores hit the wrapped layout so
            # each is one contiguous descriptor per partition.
            if "not" not in ablate:
              with (
                tc.tile_pool(name="pt", bufs=3) as pt,
                tc.tile_pool(name="pt_ps", bufs=4, space="PSUM") as ptp,
            ):
                for g in range(NTG):
                    ha = pt.tile([P, TGRP, 2, P], bf16, tag="ha")
                    nc.sync.dma_start(
                        out=ha[:], in_=hT_tbl[:, g * TGRP:(g + 1) * TGRP, :, :])
                    stg = pt.tile([P, TGRP, 384], bf16, tag="stg")
                    for s in range(TGRP):
                        ps_t = ptp.tile([P, 384], f32)
                        nc.tensor.matmul(
                            out=ps_t[:], lhsT=ha[:, s, 0, :],
                            rhs=wkv_sb[:, 0, :], start=True, stop=False,
                        )
                        nc.tensor.matmul(
                            out=ps_t[:], lhsT=ha[:, s, 1, :],
                            rhs=wkv_sb[:, 1, :], start=False, stop=True,
                        )
                        _copy(s, out=stg[:, s, :], in_=ps_t[:])
                    # stores go out on the ACT queue so they don't head-of-line
                    # block the next load on the SP queue
                    nc.scalar.dma_start(
                        out=kv_w[:, g * TGRP:(g + 1) * TGRP, :], in_=stg[:])

            # ---- phase B: blocked gather + attention ----------------------
            # idx layout: flat order [block, slice, 896]; 2 slices per gather.
            CW = BM * P // 16     # idx16 words per (block, slice) chunk
            with (
                tc.tile_pool(name="pb", bufs=3) as pb,
                tc.tile_pool(name="pb_ps", bufs=2, space="PSUM") as pbp,
                tc.tile_pool(name="po", bufs=2) as po,
            ):
                nmm = -(-BM // 2)  # two 256-wide m-tiles per PSUM bank
                for b in range(NB):
                    wvp2 = [
                        pbp.tile([P, 512], f32, tag=f"wvp{j}", name=f"wvp{j}")
                        for j in range(nmm)
                    ]
                    wvp = [
                        wvp2[m // 2][:, (m % 2) * 256:(m % 2) * 256 + 256]
                        for m in range(BM)
                    ]
                    z03 = pb.tile([P, BM, 4], f32, tag="z03", bufs=2)
                    nc.gpsimd.memset(z03[:], 0.0)
                    qb = qloc[:, b * BM:(b + 1) * BM, :]
                    for kk in range(4):
                        k0 = 2 * kk
                        gath = pb.tile([P, 2, BM, 384], bf16, tag="gath", bufs=4)
                        # SWDGE ring caps one gather at ~1024 descriptors ->
                        # two 896-idx gathers fill the 2-slice tile
                        for j in range(2):
                            w0 = (b * 8 + k0 + j) * CW
                            ("nog" in ablate) or nc.gpsimd.dma_gather(
                                out_ap=gath[:, j, :, :],
                                in_ap=kv_table[:, :],
                                idxs_ap=idx_sb[:, w0:w0 + CW],
                                num_idxs=BM * P, num_idxs_reg=BM * P,
                                elem_size=384,
                            )
                        if "nov" in ablate:
                            continue
                        # QK dot product for heads 0-3 (bf16 2x path)
                        prod = pb.tile([P, 2, BM, P], bf16, tag="prod", bufs=3)
                        nc.vector.tensor_tensor(
                            out=prod[:], in0=gath[:, :, :, 0:P],
                            in1=qb.unsqueeze(1).to_broadcast([P, 2, BM, P]),
                            op=mult)
                        dot = pb.tile([P, 2, BM, 4], f32, tag="dot", bufs=3)
                        nc.vector.tensor_reduce(
                            out=dot[:],
                            in_=prod[:].rearrange("p k m (h f) -> p k m h f", f=32),
                            axis=mybir.AxisListType.X, op=add,
                        )
                        # weight vector w = [exp-score (clipped) | jaccard]
                        w8 = pb.tile([P, 2, BM, 8], bf16, tag="w8", bufs=3)
                        nc.scalar.activation(
                            out=w8[:, :, :, 0:4], in_=dot[:],
                            func=mybir.ActivationFunctionType.Exp, scale=SCALE,
                        )
                        nc.vector.tensor_scalar(
                            out=w8[:, :, :, 0:4], in0=w8[:, :, :, 0:4],
                            scalar1=CLIP_HI, scalar2=CLIP_LO,
                            op0=mybir.AluOpType.min, op1=mybir.AluOpType.max,
                        )
                        jb = jac_sb[:, (b * 8 + k0) * BM:(b * 8 + k0 + 2) * BM]
                        nc.scalar.copy(
                            out=w8[:, :, :, 4:8],
                            in_=jb.unsqueeze(2)
                                .to_broadcast([P, 2 * BM, 4])
                                .rearrange("p (k m) h -> p k m h", k=2))
                        for j in range(2):
                            nc.gpsimd.tensor_tensor(
                                out=z03[:], in0=z03[:],
                                in1=w8[:, j, :, 0:4], op=add)
                        # scale V rows: V is stored (f,h)-minor so the weight
                        # broadcast has a packed innermost dim (DVE 2x)
                        scl = pb.tile([P, 2, BM, 256], bf16, tag="scl", bufs=3)
                        nc.vector.tensor_tensor(
                            out=scl[:].rearrange("p k m (f h) -> p k m f h", h=8),
                            in0=gath[:, :, :, P:384].rearrange(
                                "p k m (f h) -> p k m f h", h=8),
                            in1=w8[:].unsqueeze(3).to_broadcast([P, 2, BM, 32, 8]),
                            op=mult)
                        if "nom" in ablate:
                            continue
                        for j in range(2):
                            for jj in range(nmm):
                                m0, m1 = 2 * jj, min(2 * jj + 2, BM)
                                nc.tensor.matmul(
                                    out=wvp2[jj][:, :(m1 - m0) * 256],
                                    lhsT=ident[:],
                                    rhs=scl[:, j, m0:m1, :].rearrange(
                                        "p m f -> p (m f)"),
                                    start=(kk == 0 and j == 0),
                                    stop=(kk == 3 and j == 1),
                                )
                    # finalize block b: divide by z, evict, store
                    zrec = pb.tile([P, BM, 8], f32, tag="zrec", bufs=2)
                    nc.vector.reciprocal(zrec[:, :, 0:4], z03[:])
                    nc.scalar.copy(
                        out=zrec[:, :, 4:8],
                        in_=z47r[:, b * BM:(b + 1) * BM].unsqueeze(2)
                            .to_broadcast([P, BM, 4]))
                    wvs = pb.tile([P, BM, 256], f32, tag="wvs", bufs=2)
                    ost = po.tile([P, BM, 256], f32, tag="ost", bufs=2)
                    for jj in range(nmm):
                        m0, m1 = 2 * jj, min(2 * jj + 2, BM)
                        # gpsimd cannot read PSUM: evict on ACT, divide on Pool
                        nc.scalar.copy(
                            out=wvs[:, m0:m1, :],
                            in_=wvp2[jj][:, :(m1 - m0) * 256].rearrange(
                                "p (m f) -> p m f", m=m1 - m0))
                    for m in range(BM):
                        # per-m 3D APs; plain TT is the only mult Pool runs
                        nc.gpsimd.tensor_tensor(
                            out=ost[:, m, :].rearrange("p (f h) -> p f h", h=8),
                            in0=wvs[:, m, :].rearrange("p (f h) -> p f h", h=8),
                            in1=zrec[:, m, :].unsqueeze(1)
                                .to_broadcast([P, 32, 8]),
                            op=mult,
                        )
                    nc.sync.dma_start(
                        out=out_t[:, b * BM * 256:(b + 1) * BM * 256],
                        in_=ost[:].rearrange("p m f -> p (m f)"))

    nc.compile()
    return nc


# --------------------------------------------------------------------------
# host-side sharding / assembly
# --------------------------------------------------------------------------

def host_prepare(cfg: Cfg, h, src, jaccard, Wq, Wk, Wv):
    """Build the per-core input maps (layout only, no FLOPs)."""
    import ml_dtypes

    f32 = np.float32
    bf16 = ml_dtypes.bfloat16
    hT = np.ascontiguousarray(h.T.astype(f32, copy=False)).astype(bf16)

    def chunk2(w_t, width):  # [256, width] -> [128, 2, width]
        return np.ascontiguousarray(
            w_t.reshape(2, P, width).transpose(1, 0, 2)).astype(bf16)

    # V columns head-minor: col 128 + f*8 + h <- Wv row (h*32 + f)
    WvT = np.ascontiguousarray(Wv.T).astype(f32)          # [256, 256] (in, h*32+f)
    WvT = WvT.reshape(IN_DIM, 8, 32).transpose(0, 2, 1).reshape(IN_DIM, 256)
    w_kv = chunk2(np.concatenate([Wk.T.astype(f32), WvT], axis=1), 384)
    w_q = chunk2(np.ascontiguousarray(Wq.T).astype(f32), P)

    # slot s = m*128 + p  <->  node n = c*NPC + s  (s < NPC valid)
    s_grid = np.arange(cfg.M)[None, :] * P + np.arange(P)[:, None]  # [128, M]
    valid = s_grid < cfg.NPC

    in_maps = []
    for c in range(cfg.NC):
        base = c * cfg.NPC
        src_mat = np.zeros((8, cfg.NPAD), dtype=np.int64)
        jacv = np.ones((8, P, cfg.M), dtype=f32)
        for k in range(8):
            e = k * cfg.N + base + np.clip(s_grid, 0, cfg.NPC - 1)
            sv = np.where(valid, src[e], -1)               # [128, M]; -1 pad
            src_mat[k] = sv.T.reshape(-1)                  # slot order m*128+p
            jacv[k] = np.where(valid, jaccard[e], 1.0)
        # block-major jac: [p, (block, slice, m)]
        jac = np.ascontiguousarray(
            jacv.reshape(8, P, cfg.NB, cfg.BM).transpose(1, 2, 0, 3)
        ).reshape(P, 8 * cfg.M)
        uniq = np.unique(src_mat[src_mat >= 0])
        if len(uniq) > cfg.TBLC:
            raise _CompactionOverflow(len(uniq))
        renum = np.searchsorted(uniq, np.maximum(src_mat, uniq[0]))
        renum = np.where(src_mat >= 0, renum, 0).astype(np.int64)  # [8, NPAD]
        # wrapped table id: row r lands at DRAM row (r%128)*C + r//128
        widx = ((renum % P) * cfg.C + renum // P).astype(np.int16)
        # flat gather order [block, slice, 896]; packed [i%16, i//16]
        flat = widx.reshape(8, cfg.NB, cfg.BM * P).transpose(1, 0, 2).reshape(-1)
        idx16 = np.tile(
            flat.reshape(-1, 16).T,            # [16, total/16]
            (8, 1),
        )
        idx16 = np.ascontiguousarray(idx16)

        # wrapped h table: [p, chunk, half, node]
        uniq_pad = np.zeros(cfg.TBLC, dtype=np.int64)
        uniq_pad[:len(uniq)] = uniq
        sub = np.asarray(hT[:, uniq_pad])                  # [256, TBLC]
        hT_tbl = np.ascontiguousarray(
            sub.reshape(2, P, cfg.C, P).transpose(1, 2, 0, 3))
        hT_own = np.zeros((IN_DIM, cfg.NPAD), dtype=bf16)
        span = min(cfg.NPAD, cfg.N - base)
        hT_own[:, :span] = hT[:, base:base + span]
        in_maps.append({
            "hT_tbl": hT_tbl,
            "hT_own": hT_own,
            "w_kv": w_kv,
            "w_q": w_q,
            "idx": idx16,
            "jac": jac.astype(bf16),
        })
    return in_maps


class _CompactionOverflow(Exception):
    pass


def assemble_output(cfg: Cfg, results):
    out = np.empty((cfg.N, NUM_HEADS, OUT_DIM), dtype=np.float32)
    for c, r in enumerate(results):
        # device layout: [p, m, f*8 + h] -> [node, h, f]
        shard = r["out"].reshape(P, cfg.M, OUT_DIM, NUM_HEADS)
        shard = shard.transpose(1, 0, 3, 2).reshape(cfg.NPAD, NUM_HEADS, OUT_DIM)
        out[c * cfg.NPC:(c + 1) * cfg.NPC] = shard[:cfg.NPC]
    return out


# --------------------------------------------------------------------------
# numpy fallback (used only if inputs don't match the spec'd structure)
# --------------------------------------------------------------------------

def _numpy_reference(h, src, dst, jaccard, Wq, bq, Wk, bk, Wv, bv):
    N = h.shape[0]
    E = src.shape[0]
    h = h.astype(np.float32)
    Qh = (h @ Wq.T + bq).reshape(N, H2, OUT_DIM)
    Kh = (h @ Wk.T + bk).reshape(N, H2, OUT_DIM)
    Vh = (h @ Wv.T + bv).reshape(N, NUM_HEADS, OUT_DIM)
    score = np.sum(Kh[src] * Qh[dst], axis=-1, keepdims=True)
    score = np.exp(np.clip(score / np.sqrt(np.float32(OUT_DIM)), -5.0, 5.0))
    jac = np.broadcast_to(jaccard[:, None, None], (E, H2, 1))
    score_new = np.concatenate([score, jac], axis=1).astype(np.float32)
    contrib = (Vh[src] * score_new).astype(np.float32)
    wV = np.zeros((N, NUM_HEADS, OUT_DIM), dtype=np.float32)
    z = np.zeros((N, NUM_HEADS, 1), dtype=np.float32)
    np.add.at(wV, dst, contrib)
    np.add.at(z, dst, score_new)
    return wV / z


# --------------------------------------------------------------------------
# entry point
# --------------------------------------------------------------------------

_PROGRAM_CACHE = {}


def _get_program(cfg: Cfg):
    key = (cfg.N, cfg.E, cfg.NC)
    if key not in _PROGRAM_CACHE:
        _PROGRAM_CACHE[key] = build_program(cfg)
    return _PROGRAM_CACHE[key]


def _structure_ok(h, src, dst, jaccard, Wq, bq, Wk, bk, Wv, bv):
    if h.shape != (N_NODES, IN_DIM) or src.shape != (N_EDGES,):
        return False
    if Wq.shape != (H2 * OUT_DIM, IN_DIM) or Wv.shape != (NUM_HEADS * OUT_DIM, IN_DIM):
        return False
    if np.any(bq) or np.any(bk) or np.any(bv):
        return False
    if not np.array_equal(
            np.asarray(dst, dtype=np.int64),
            np.arange(N_EDGES, dtype=np.int64) % N_NODES):
        return False
    if src.min() < 0 or src.max() >= N_NODES:
        return False
    return True


def run_on_hw(inputs):
    from concourse.bass2jax import run_bass_via_pjrt

    cfg = FULL_CFG
    nc = _get_program(cfg)
    in_maps = host_prepare(
        cfg, inputs["h"], inputs["src"], inputs["jaccard"],
        inputs["Wq"], inputs["Wk"], inputs["Wv"])
    results = run_bass_via_pjrt(nc, in_maps, n_cores=cfg.NC)
    return assemble_output(cfg, results), results


def kernel(**inputs) -> np.ndarray:
    args = {k: np.asarray(v) for k, v in inputs.items()}
    if not _structure_ok(**args):
        return _numpy_reference(**args)
    try:
        out, _ = run_on_hw(args)
    except _CompactionOverflow:
        return _numpy_reference(**args)
    return out


if __name__ == "__main__":
    print("building full program...")
    nc = _get_program(FULL_CFG)
    print("ok")


# revision 13
# speedup vs baseline: 2.2282x; 2.2282x over previous
"""Trainium2 Bass kernel for nn_MultiHeadAttentionLayer (GNN message passing).

Contract: kernel(**inputs) takes the FULL unsharded inputs (as produced by
setup_inputs()) and returns the FULL output [N, H, D] float32.

Strategy (8 NeuronCores, SPMD, no collectives):
  - dst == arange(E) % N, so node n receives exactly 8 edges e = k*N + n.
    Destination nodes are sharded across cores; each core owns N/8 nodes and
    all 8 incoming edges per node, so segment_sum is a dense 8-step PSUM
    accumulation (no scatter).
  - Each core renumbers the src ids of its edges into a compacted table of
    < 32768 unique rows (int16 gather indices).  On device it projects that
    node set through [Wk|Wv] into a 768B-per-row bf16 [K|V] table in DRAM,
    laid out "wrapped" ([128, C, 384]; row r at partition r%128, chunk
    r//128) so table stores batch 12 row-tiles into a single DMA.
  - V columns are stored head-minor ((f,h) order) so the per-edge score
    multiply has a packed innermost dim and runs in the DVE 2x mode with a
    stride-0 broadcast of the per-(edge,head) weight vector; output heads are
    un-transposed on the host for free.
  - Per (node-block, 2-slice chunk): one 768B-row dma_gather, score =
    clip(exp(dot(K,Q)*scale)) (exp-then-clip == clip-then-exp by
    monotonicity, so no pre-clip pass), weight vector w = [exp-score | jac],
    scl = V * w, and identity-matmul PSUM accumulation over the 8 slices.
    Work is spread: DVE (prod/reduce/scale), ACT (exp, broadcast fills),
    Pool (gathers, clip, z-accum, final divide), PE (accumulate).

Host-side work is limited to layout (transpose/pad/unique/renumber tables)
and the final concatenation; all FLOPs happen on device.
"""

import sys

import numpy as np

for _p in ("/opt/trn_rl_repo",):
    if _p not in sys.path:
        sys.path.insert(0, _p)

# --- problem constants (hardcoded per spec; kernel.py must be self-contained)
N_NODES = 50000
N_EDGES = 400000
IN_DIM = 256
OUT_DIM = 32
NUM_HEADS = 8
H2 = NUM_HEADS // 2
N_CORES = 8
P = 128

SCALE = float(1.0 / np.sqrt(np.float32(32.0)))
CLIP_LO = float(np.exp(np.float32(-5.0)))
CLIP_HI = float(np.exp(np.float32(5.0)))
MAX_TBLC = 32256  # compacted-table cap (int16 indices; 252 * 128)


class Cfg:
    def __init__(self, n_nodes=N_NODES, n_edges=N_EDGES, n_cores=N_CORES):
        assert n_edges == 8 * n_nodes
        self.N = n_nodes
        self.E = n_edges
        self.NC = n_cores
        assert n_nodes % n_cores == 0
        self.NPC = n_nodes // n_cores          # nodes per core
        self.M = -(-self.NPC // P)             # 128-node tiles per core
        self.NPAD = self.M * P                 # padded nodes per core
        assert self.NPAD % 16 == 0
        self.W = self.NPAD // 16               # idx16 words per slice
        self.BM = 7 if self.M % 7 == 0 else 1  # m-tiles per node block
        self.NB = self.M // self.BM            # node blocks
        # compacted table rows (upper bound on per-core unique src count)
        self.TBLC = min(-(-n_nodes // 512) * 512, MAX_TBLC)
        assert self.TBLC % P == 0
        self.C = self.TBLC // P                # wrapped-table chunks
        self.TGRP = 12 if self.C % 12 == 0 else (4 if self.C % 4 == 0 else 1)
        self.NTG = self.C // self.TGRP         # table store groups


FULL_CFG = Cfg()


# --------------------------------------------------------------------------
# device program
# --------------------------------------------------------------------------

def build_program(cfg: Cfg, repeat: int = 1, ablate: str = ""):
    import concourse.bacc as bacc
    import concourse.mybir as mybir
    import concourse.tile as tile
    from concourse.masks import make_identity

    f32 = mybir.dt.float32
    bf16 = mybir.dt.bfloat16
    i16 = mybir.dt.int16
    M, BM, NB = cfg.M, cfg.BM, cfg.NB
    TGRP, NTG = cfg.TGRP, cfg.NTG

    nc = bacc.Bacc(
        "TRN2",
        target_bir_lowering=False,
        debug=False,
        enable_asserts=False,
        num_devices=cfg.NC,
    )

    # hT_tbl wrapped: [p, chunk, half, node] (see host_prepare)
    hT_tbl = nc.dram_tensor("hT_tbl", [P, cfg.C, 2, P], bf16, kind="ExternalInput")
    hT_own = nc.dram_tensor("hT_own", [IN_DIM, cfg.NPAD], bf16, kind="ExternalInput")
    w_kv = nc.dram_tensor("w_kv", [P, 2, 384], bf16, kind="ExternalInput")
    w_q = nc.dram_tensor("w_q", [P, 2, P], bf16, kind="ExternalInput")
    idx_t = nc.dram_tensor("idx", [P, 8 * cfg.W], i16, kind="ExternalInput")
    jac_t = nc.dram_tensor("jac", [P, 8 * M], bf16, kind="ExternalInput")
    out_t = nc.dram_tensor("out", [P, M * 256], f32, kind="ExternalOutput")

    mult = mybir.AluOpType.mult
    add = mybir.AluOpType.add


    with tile.TileContext(nc) as tc:
        with (
            tc.tile_pool(name="dram", bufs=2, space="DRAM") as dram,
            tc.tile_pool(name="const", bufs=1) as const,
        ):
          for _rep in range(repeat):
            kv_table = dram.tile([cfg.TBLC, 384], bf16)
            kv_w = kv_table[:].rearrange("(p c) f -> p c f", p=P)  # wrapped view

            wkv_sb = const.tile([P, 2, 384], bf16)
            wq_sb = const.tile([P, 2, P], bf16)
            idx_sb = const.tile([P, 8 * cfg.W], i16)
            jac_sb = const.tile([P, 8 * M], bf16)
            qloc = const.tile([P, M, P], bf16)
            z47r = const.tile([P, M], f32)
            ident = const.tile([P, P], bf16)

            nc.sync.dma_start(out=wkv_sb[:], in_=w_kv[:])
            nc.sync.dma_start(out=wq_sb[:], in_=w_q[:])
            nc.sync.dma_start(out=idx_sb[:], in_=idx_t[:])
            nc.sync.dma_start(out=jac_sb[:], in_=jac_t[:])
            make_identity(nc, ident[:])

            # z for jaccard heads: one strided reduce over the 8 slices
            # jac layout is block-major: [p, (b, k, m)]
            nc.vector.tensor_reduce(
                out=z47r[:].rearrange("p (b m) -> p b m", m=BM),
                in_=jac_sb[:].rearrange("p (b k m) -> p b m k", k=8, m=BM),
                axis=mybir.AxisListType.X, op=add)
            nc.vector.reciprocal(z47r[:], z47r[:])

            def _copy(i, out, in_):
                # PSUM -> SBUF eviction; gpsimd cannot read PSUM on HW
                if i % 2 == 0:
                    nc.scalar.copy(out=out, in_=in_)
                else:
                    nc.vector.tensor_copy(out=out, in_=in_)

            # ---- phase A: Q projection of own shard -----------------------
            with (
                tc.tile_pool(name="pa", bufs=1) as pa,
                tc.tile_pool(name="pa_ps", bufs=2, space="PSUM") as pap,
            ):
                hto = pa.tile([P, 2, cfg.NPAD], bf16)
                nc.sync.dma_start(out=hto[:, 0, :], in_=hT_own[0:P, :])
                nc.sync.dma_start(out=hto[:, 1, :], in_=hT_own[P:2 * P, :])
                for t in range(M):
                    ps_q = pap.tile([P, P], f32)
                    nc.tensor.matmul(
                        out=ps_q[:], lhsT=hto[:, 0, t * P:(t + 1) * P],
                        rhs=wq_sb[:, 0, :], start=True, stop=False,
                    )
                    nc.tensor.matmul(
                        out=ps_q[:], lhsT=hto[:, 1, t * P:(t + 1) * P],
                        rhs=wq_sb[:, 1, :], start=False, stop=True,
                    )
                    _copy(t, out=qloc[:, t, :], in_=ps_q[:])

            # ---- phase T: compacted K|V bf16 table projection -------------
            # 12 row-tiles per load/store; stores hit the wrapped layout so
            # each is one contiguous descriptor per partition.
            if "not" not in ablate:
              with (
                tc.tile_pool(name="pt", bufs=3) as pt,
                tc.tile_pool(name="pt_ps", bufs=4, space="PSUM") as ptp,
            ):
                for g in range(NTG):
                    ha = pt.tile([P, TGRP, 2, P], bf16, tag="ha")
                    nc.sync.dma_start(
                        out=ha[:], in_=hT_tbl[:, g * TGRP:(g + 1) * TGRP, :, :])
                    stg = pt.tile([P, TGRP, 384], bf16, tag="stg")
                    for s in range(TGRP):
                        ps_t = ptp.tile([P, 384], f32)
                        nc.tensor.matmul(
                            out=ps_t[:], lhsT=ha[:, s, 0, :],
                            rhs=wkv_sb[:, 0, :], start=True, stop=False,
                        )
                        nc.tensor.matmul(
                            out=ps_t[:], lhsT=ha[:, s, 1, :],
                            rhs=wkv_sb[:, 1, :], start=False, stop=True,
                        )
                        _copy(s, out=stg[:, s, :], in_=ps_t[:])
                    # stores go out on the ACT queue so they don't head-of-line
                    # block the next load on the SP queue
                    nc.scalar.dma_start(
                        out=kv_w[:, g * TGRP:(g + 1) * TGRP, :], in_=stg[:])

            # ---- phase B: blocked gather + attention ----------------------
            # idx layout: flat order [block, slice, 896]; 2 slices per gather.
            CW = BM * P // 16     # idx16 words per (block, slice) chunk
            with (
                tc.tile_pool(name="pb", bufs=3) as pb,
                tc.tile_pool(name="pb_ps", bufs=2, space="PSUM") as pbp,
                tc.tile_pool(name="po", bufs=2) as po,
            ):
                nmm = -(-BM // 2)  # two 256-wide m-tiles per PSUM bank
                for b in range(NB):
                    wvp2 = [
                        pbp.tile([P, 512], f32, tag=f"wvp{j}", name=f"wvp{j}")
                        for j in range(nmm)
                    ]
                    wvp = [
                        wvp2[m // 2][:, (m % 2) * 256:(m % 2) * 256 + 256]
                        for m in range(BM)
                    ]
                    z03 = pb.tile([P, BM, 4], f32, tag="z03", bufs=2)
                    nc.vector.memset(z03[:], 0.0)
                    qb = qloc[:, b * BM:(b + 1) * BM, :]
                    for kk in range(4):
                        k0 = 2 * kk
                        gath = pb.tile([P, 2, BM, 384], bf16, tag="gath", bufs=4)
                        # SWDGE ring caps one gather at ~1024 descriptors ->
                        # two 896-idx gathers fill the 2-slice tile
                        for j in range(2):
                            w0 = (b * 8 + k0 + j) * CW
                            ("nog" in ablate) or nc.gpsimd.dma_gather(
                                out_ap=gath[:, j, :, :],
                                in_ap=kv_table[:, :],
                                idxs_ap=idx_sb[:, w0:w0 + CW],
                                num_idxs=BM * P, num_idxs_reg=BM * P,
                                elem_size=384,
                            )
                        if "nov" in ablate:
                            continue
                        # QK dot product for heads 0-3 (bf16 2x path)
                        prod = pb.tile([P, 2, BM, P], bf16, tag="prod", bufs=3)
                        nc.vector.tensor_tensor(
                            out=prod[:], in0=gath[:, :, :, 0:P],
                            in1=qb.unsqueeze(1).to_broadcast([P, 2, BM, P]),
                            op=mult)
                        dot = pb.tile([P, 2, BM, 4], f32, tag="dot", bufs=3)
                        nc.vector.tensor_reduce(
                            out=dot[:],
                            in_=prod[:].rearrange("p k m (h f) -> p k m h f", f=32),
                            axis=mybir.AxisListType.X, op=add,
                        )
                        # weight vector w = [exp-score (clipped) | jaccard]
                        w8 = pb.tile([P, 2, BM, 8], bf16, tag="w8", bufs=3)
                        nc.scalar.activation(
                            out=w8[:, :, :, 0:4], in_=dot[:],
                            func=mybir.ActivationFunctionType.Exp, scale=SCALE,
                        )
                        nc.vector.tensor_scalar(
                            out=w8[:, :, :, 0:4], in0=w8[:, :, :, 0:4],
                            scalar1=CLIP_HI, scalar2=CLIP_LO,
                            op0=mybir.AluOpType.min, op1=mybir.AluOpType.max,
                        )
                        jb = jac_sb[:, (b * 8 + k0) * BM:(b * 8 + k0 + 2) * BM]
                        nc.scalar.copy(
                            out=w8[:, :, :, 4:8],
                            in_=jb.unsqueeze(2)
                                .to_broadcast([P, 2 * BM, 4])
                                .rearrange("p (k m) h -> p k m h", k=2))
                        for j in range(2):
                            nc.vector.tensor_tensor(
                                out=z03[:], in0=z03[:],
                                in1=w8[:, j, :, 0:4], op=add)
                        # scale V rows: V is stored (f,h)-minor so the weight
                        # broadcast has a packed innermost dim (DVE 2x)
                        scl = pb.tile([P, 2, BM, 256], bf16, tag="scl", bufs=3)
                        nc.vector.tensor_tensor(
                            out=scl[:].rearrange("p k m (f h) -> p k m f h", h=8),
                            in0=gath[:, :, :, P:384].rearrange(
                                "p k m (f h) -> p k m f h", h=8),
                            in1=w8[:].unsqueeze(3).to_broadcast([P, 2, BM, 32, 8]),
                            op=mult)
                        if "nom" in ablate:
                            continue
                        for j in range(2):
                            for jj in range(nmm):
                                m0, m1 = 2 * jj, min(2 * jj + 2, BM)
                                nc.tensor.matmul(
                                    out=wvp2[jj][:, :(m1 - m0) * 256],
                                    lhsT=ident[:],
                                    rhs=scl[:, j, m0:m1, :].rearrange(
                                        "p m f -> p (m f)"),
                                    start=(kk == 0 and j == 0),
                                    stop=(kk == 3 and j == 1),
                                )
                    # finalize block b: divide by z, evict, store
                    if "nov" in ablate:
                        ost = po.tile([P, BM, 256], f32, tag="ost", bufs=2)
                        nc.vector.memset(ost[:], 0.0)
                        nc.sync.dma_start(
                            out=out_t[:, b * BM * 256:(b + 1) * BM * 256],
                            in_=ost[:].rearrange("p m f -> p (m f)"))
                        continue
                    zrec = pb.tile([P, BM, 8], f32, tag="zrec", bufs=2)
                    nc.vector.reciprocal(zrec[:, :, 0:4], z03[:])
                    nc.scalar.copy(
                        out=zrec[:, :, 4:8],
                        in_=z47r[:, b * BM:(b + 1) * BM].unsqueeze(2)
                            .to_broadcast([P, BM, 4]))
                    wvs = pb.tile([P, BM, 256], f32, tag="wvs", bufs=2)
                    ost = po.tile([P, BM, 256], f32, tag="ost", bufs=2)
                    for jj in range(nmm):
                        m0, m1 = 2 * jj, min(2 * jj + 2, BM)
                        # gpsimd cannot read PSUM: evict on ACT, divide on Pool
                        nc.scalar.copy(
                            out=wvs[:, m0:m1, :],
                            in_=wvp2[jj][:, :(m1 - m0) * 256].rearrange(
                                "p (m f) -> p m f", m=m1 - m0))
                    for m in range(BM):
                        nc.vector.tensor_tensor(
                            out=ost[:, m, :].rearrange("p (f h) -> p f h", h=8),
                            in0=wvs[:, m, :].rearrange("p (f h) -> p f h", h=8),
                            in1=zrec[:, m, :].unsqueeze(1)
                                .to_broadcast([P, 32, 8]),
                            op=mult,
                        )
                    nc.sync.dma_start(
                        out=out_t[:, b * BM * 256:(b + 1) * BM * 256],
                        in_=ost[:].rearrange("p m f -> p (m f)"))

    nc.compile()
    return nc


# --------------------------------------------------------------------------
# host-side sharding / assembly
# --------------------------------------------------------------------------

def host_prepare(cfg: Cfg, h, src, jaccard, Wq, Wk, Wv):
    """Build the per-core input maps (layout only, no FLOPs)."""
    import ml_dtypes

    f32 = np.float32
    bf16 = ml_dtypes.bfloat16
    hT = np.ascontiguousarray(h.T.astype(f32, copy=False)).astype(bf16)

    def chunk2(w_t, width):  # [256, width] -> [128, 2, width]
        return np.ascontiguousarray(
            w_t.reshape(2, P, width).transpose(1, 0, 2)).astype(bf16)

    # V columns head-minor: col 128 + f*8 + h <- Wv row (h*32 + f)
    WvT = np.ascontiguousarray(Wv.T).astype(f32)          # [256, 256] (in, h*32+f)
    WvT = WvT.reshape(IN_DIM, 8, 32).transpose(0, 2, 1).reshape(IN_DIM, 256)
    w_kv = chunk2(np.concatenate([Wk.T.astype(f32), WvT], axis=1), 384)
    w_q = chunk2(np.ascontiguousarray(Wq.T).astype(f32), P)

    # slot s = m*128 + p  <->  node n = c*NPC + s  (s < NPC valid)
    s_grid = np.arange(cfg.M)[None, :] * P + np.arange(P)[:, None]  # [128, M]
    valid = s_grid < cfg.NPC

    in_maps = []
    for c in range(cfg.NC):
        base = c * cfg.NPC
        src_mat = np.zeros((8, cfg.NPAD), dtype=np.int64)
        jacv = np.ones((8, P, cfg.M), dtype=f32)
        for k in range(8):
            e = k * cfg.N + base + np.clip(s_grid, 0, cfg.NPC - 1)
            sv = np.where(valid, src[e], -1)               # [128, M]; -1 pad
            src_mat[k] = sv.T.reshape(-1)                  # slot order m*128+p
            jacv[k] = np.where(valid, jaccard[e], 1.0)
        # block-major jac: [p, (block, slice, m)]
        jac = np.ascontiguousarray(
            jacv.reshape(8, P, cfg.NB, cfg.BM).transpose(1, 2, 0, 3)
        ).reshape(P, 8 * cfg.M)
        uniq = np.unique(src_mat[src_mat >= 0])
        if len(uniq) > cfg.TBLC:
            raise _CompactionOverflow(len(uniq))
        renum = np.searchsorted(uniq, np.maximum(src_mat, uniq[0]))
        renum = np.where(src_mat >= 0, renum, 0).astype(np.int64)  # [8, NPAD]
        # wrapped table id: row r lands at DRAM row (r%128)*C + r//128
        widx = ((renum % P) * cfg.C + renum // P).astype(np.int16)
        # flat gather order [block, slice, 896]; packed [i%16, i//16]
        flat = widx.reshape(8, cfg.NB, cfg.BM * P).transpose(1, 0, 2).reshape(-1)
        idx16 = np.tile(
            flat.reshape(-1, 16).T,            # [16, total/16]
            (8, 1),
        )
        idx16 = np.ascontiguousarray(idx16)

        # wrapped h table: [p, chunk, half, node]
        uniq_pad = np.zeros(cfg.TBLC, dtype=np.int64)
        uniq_pad[:len(uniq)] = uniq
        sub = np.asarray(hT[:, uniq_pad])                  # [256, TBLC]
        hT_tbl = np.ascontiguousarray(
            sub.reshape(2, P, cfg.C, P).transpose(1, 2, 0, 3))
        hT_own = np.zeros((IN_DIM, cfg.NPAD), dtype=bf16)
        span = min(cfg.NPAD, cfg.N - base)
        hT_own[:, :span] = hT[:, base:base + span]
        in_maps.append({
            "hT_tbl": hT_tbl,
            "hT_own": hT_own,
            "w_kv": w_kv,
            "w_q": w_q,
            "idx": idx16,
            "jac": jac.astype(bf16),
        })
    return in_maps


class _CompactionOverflow(Exception):
    pass


def assemble_output(cfg: Cfg, results):
    out = np.empty((cfg.N, NUM_HEADS, OUT_DIM), dtype=np.float32)
    for c, r in enumerate(results):
        # device layout: [p, m, f*8 + h] -> [node, h, f]
        shard = r["out"].reshape(P, cfg.M, OUT_DIM, NUM_HEADS)
        shard = shard.transpose(1, 0, 3, 2).reshape(cfg.NPAD, NUM_HEADS, OUT_DIM)
        out[c * cfg.NPC:(c + 1) * cfg.NPC] = shard[:cfg.NPC]
    return out


# --------------------------------------------------------------------------
# numpy fallback (used only if inputs don't match the spec'd structure)
# --------------------------------------------------------------------------

def _numpy_reference(h, src, dst, jaccard, Wq, bq, Wk, bk, Wv, bv):
    N = h.shape[0]
    E = src.shape[0]
    h = h.astype(np.float32)
    Qh = (h @ Wq.T + bq).reshape(N, H2, OUT_DIM)
    Kh = (h @ Wk.T + bk).reshape(N, H2, OUT_DIM)
    Vh = (h @ Wv.T + bv).reshape(N, NUM_HEADS, OUT_DIM)
    score = np.sum(Kh[src] * Qh[dst], axis=-1, keepdims=True)
    score = np.exp(np.clip(score / np.sqrt(np.float32(OUT_DIM)), -5.0, 5.0))
    jac = np.broadcast_to(jaccard[:, None, None], (E, H2, 1))
    score_new = np.concatenate([score, jac], axis=1).astype(np.float32)
    contrib = (Vh[src] * score_new).astype(np.float32)
    wV = np.zeros((N, NUM_HEADS, OUT_DIM), dtype=np.float32)
    z = np.zeros((N, NUM_HEADS, 1), dtype=np.float32)
    np.add.at(wV, dst, contrib)
    np.add.at(z, dst, score_new)
    return wV / z


# --------------------------------------------------------------------------
# entry point
# --------------------------------------------------------------------------

_PROGRAM_CACHE = {}


def _get_program(cfg: Cfg):
    key = (cfg.N, cfg.E, cfg.NC)
    if key not in _PROGRAM_CACHE:
        _PROGRAM_CACHE[key] = build_program(cfg)
    return _PROGRAM_CACHE[key]


def _structure_ok(h, src, dst, jaccard, Wq, bq, Wk, bk, Wv, bv):
    if h.shape != (N_NODES, IN_DIM) or src.shape != (N_EDGES,):
        return False
    if Wq.shape != (H2 * OUT_DIM, IN_DIM) or Wv.shape != (NUM_HEADS * OUT_DIM, IN_DIM):
        return False
    if np.any(bq) or np.any(bk) or np.any(bv):
        return False
    if not np.array_equal(
            np.asarray(dst, dtype=np.int64),
            np.arange(N_EDGES, dtype=np.int64) % N_NODES):
        return False
    if src.min() < 0 or src.max() >= N_NODES:
        return False
    return True


def run_on_hw(inputs):
    from concourse.bass2jax import run_bass_via_pjrt

    cfg = FULL_CFG
    nc = _get_program(cfg)
    in_maps = host_prepare(
        cfg, inputs["h"], inputs["src"], inputs["jaccard"],
        inputs["Wq"], inputs["Wk"], inputs["Wv"])
    results = run_bass_via_pjrt(nc, in_maps, n_cores=cfg.NC)
    return assemble_output(cfg, results), results


def kernel(**inputs) -> np.ndarray:
    args = {k: np.asarray(v) for k, v in inputs.items()}
    if not _structure_ok(**args):
        return _numpy_reference(**args)
    try:
        out, _ = run_on_hw(args)
    except _CompactionOverflow:
        return _numpy_reference(**args)
    return out


if __name__ == "__main__":
    print("building full program...")
    nc = _get_program(FULL_CFG)
    print("ok")
